# revision 50
# baseline (speedup 1.0000x reference)
"""CFD-GCN Trainium2 kernel: 6-layer GCN on a batched random mesh graph +
KNN interpolation, distributed over 8 NeuronCores.

Each sample (4 total) is split across a PAIR of cores: core 2s owns node
tiles 0..78, core 2s+1 owns 79..157 (79 tiles of 128 nodes). Dense (h@W),
KNN selection, interpolation and aggregation all run on the owned half.
Per layer, a 2-chunk pair AllGather publishes the dense output g; each
tile's edge gather is split into an own-half gather (reads local g_half,
no collective wait) and a peer-half gather (reads g_full, waits the
AllGather). Self-loops ride in the own-gather slots. Descriptors are
generated with prepare_only + trigger_dma on 4 SWDGE queues. g_half and
g_full ping-pong across layers so dense l overlaps aggregation l-1
(per-tile h-write marks instead of a layer barrier); the dense
psum->SBUF copy runs on the vector engine to stay clear of the scalar
queue.

Self-contained: hardcodes all shapes; the slice profiles (own/peer slots
per dest tile) are derived from the inputs on first call and baked into
the program. kernel(**inputs) -> np.ndarray [80000, 3].
"""
import sys

sys.path.insert(0, "/opt/trn_rl_repo")

import numpy as np
import ml_dtypes

from concourse import bass, bacc
from concourse.bass_utils import run_bass_kernel_spmd
import concourse.mybir as mybir
from contextlib import ExitStack

f32, bf16 = mybir.dt.float32, mybir.dt.bfloat16
i16, u16 = mybir.dt.int16, mybir.dt.uint16
ALU = mybir.AluOpType
ACTF = mybir.ActivationFunctionType
bfnp = ml_dtypes.bfloat16

# ---------------- problem constants ----------------
B, NF, NC, H, D_IN, OUT = 4, 20000, 2000, 512, 5, 3
E_PER = 6 * NF
NT = 158                      # global node tiles per sample
NTH = 79                      # node tiles per core (half sample)
NPAD = NT * 128               # 20224
NPADH = NTH * 128             # 10112
NCPAD = 2048                  # padded coarse count
RANGE_T = 16                  # node tiles per hT transpose-load range
ICH = 8                       # interp gather chunk (tiles)
N_CORES = 8
GB = 4                        # agg ring depth (gather bufs)
AGCH = [(0, 40), (40, 79)]    # AllGather chunks (tiles)
PHASE = 99                    # debug: truncate program after checkpoint N

LAYERS = [
    dict(kc6=True, fy=False, relu=True, e2=False),   # pre0
    dict(kc6=False, fy=False, relu=True, e2=False),  # pre1
    dict(kc6=False, fy=False, relu=True, e2=False),  # pre2
    dict(kc6=False, fy=True, relu=True, e2=False),   # end0
    dict(kc6=False, fy=False, relu=True, e2=False),  # end1
    dict(kc6=False, fy=False, relu=False, e2=True),  # end2
]

RG_PAIRS = [[0, 1], [2, 3], [4, 5], [6, 7]]


def _ranges():
    r, t0 = [], 0
    while t0 < NTH:
        r.append((t0, min(RANGE_T, NTH - t0)))
        t0 += RANGE_T
    return r


def build_program(Po, Pp):
    """Po/Pp: per-local-tile own/peer slice counts, identical on all cores."""
    P = Po + Pp
    SOFF = np.concatenate([[0], np.cumsum(P)]).astype(int)      # slice offs
    SOFFC = (SOFF * 136).astype(int)                            # sgt col offs
    MAXP = int(P.max())

    nc = bacc.Bacc(num_devices=N_CORES, num_swdge_queues=4)

    Din = {}
    def din(name, shape, dt):
        Din[name] = nc.declare_dram_parameter(name, list(shape), dt, isOutput=False)
    def dout(name, shape, dt):
        Din[name] = nc.declare_dram_parameter(name, list(shape), dt, isOutput=True)

    din("xT3", (3, NPADH), f32)
    din("cxT3", (3, NCPAD), f32)
    din("negf2", (128, NTH), f32)
    din("h0T", (6, NPADH), bf16)
    din("W0", (6, H), bf16)
    din("W1", (128, 4, H), bf16)      # p-major k-chunked
    din("W2", (128, 4, H), bf16)
    din("W3a", (128, 4, H), bf16)
    din("W3b", (3, H), bf16)
    din("W4", (128, 4, H), bf16)
    din("W5", (128, 4, 128), bf16)
    din("brows", (1, 6, H), bf16)
    din("ones1", (1, 128), bf16)
    din("identf", (128, 128), f32)
    din("sgt", (128, int(SOFFC[-1])), bf16)   # per tile: S p-major | idxs
    din("ctab", (NCPAD, 128), bf16)

    g_half = [nc.dram_tensor(f"g_half{i}", [NPADH, H], bf16) for i in range(2)]
    g_full = [nc.dram_tensor(f"g_full{i}", [NPAD, H], bf16) for i in range(2)]
    g2_half = nc.dram_tensor("g2_half", [NPADH, 128], bf16)
    g2_full = nc.dram_tensor("g2_full", [NPAD, 128], bf16)
    h_d = nc.dram_tensor("h_d", [NPADH, H], bf16)
    fy_d = nc.dram_tensor("fy_d", [3, NPADH], bf16)
    dout("out_nm", (NPADH, 128), f32)

    es = ExitStack()
    def sb(name, shape, dt):
        return es.enter_context(nc.sbuf_tensor(name, list(shape), dt))
    def psum(name, shape, dt):
        return es.enter_context(nc.psum_tensor(name, list(shape), dt))

    xt_s = [sb(f"xt_s{i}", (3, 128), f32) for i in range(2)]
    cxT3_s = sb("cxT3_s", (3, NCPAD), f32)
    negf2_s = sb("negf2_s", (128, NTH), f32)
    h0_s = [sb(f"h0_s{i}", (6, 128), bf16) for i in range(2)]
    W0_s = sb("W0_s", (6, H), bf16)
    W1_s = sb("W1_s", (128, 4, H), bf16)
    W2_s = sb("W2_s", (128, 4, H), bf16)
    W3a_s = sb("W3a_s", (128, 4, H), bf16)
    W3b_s = sb("W3b_s", (3, H), bf16)
    W4_s = sb("W4_s", (128, 4, H), bf16)
    W5_s = sb("W5_s", (128, 4, 128), bf16)
    brows_s = sb("brows_s", (1, 6, H), bf16)
    ones1_s = sb("ones1_s", (1, 128), bf16)
    identf_s = sb("identf_s", (128, 128), f32)

    hT_s = [sb(f"hT_s{i}", (128, 4, RANGE_T * 128), bf16) for i in range(2)]
    gsb_s = [sb(f"gsb_s{i}", (128, H), bf16) for i in range(4)]
    hsb_s = [sb(f"hsb_s{i}", (128, H), bf16) for i in range(4)]
    osb_s = [sb(f"osb_s{i}", (128, 128), f32) for i in range(2)]
    gath_s = [sb(f"gath_s{i}", (128, MAXP, H), bf16) for i in range(GB)]
    gath2_s = [sb(f"gath2_s{i}", (128, MAXP, 128), bf16) for i in range(GB)]
    sgt_s = [sb(f"sgt_s{i}", (128, MAXP * 136), bf16) for i in range(GB)]

    nd2_s = [sb(f"nd2_s{i}", (128, NCPAD), f32) for i in range(2)]
    bm_s = sb("bm_s", (128, 8, NTH), f32)
    bi_s = sb("bi_s", (128, 8, NTH), u16)
    d2c_s = sb("d2c_s", (128, 3, NTH), f32)
    w_s = sb("w_s", (128, 3, NTH), f32)
    wsum_s = sb("wsum_s", (128, NTH), f32)
    rs_s = sb("rs_s", (128, NTH), f32)
    wnb_s = sb("wnb_s", (128, 3, NTH), f32)
    wrap_s = sb("wrap_s", (128, 3, NTH, 8), u16)
    gk_s = [[sb(f"gk_s{k}_{i}", (128, ICH, 128), bf16) for i in range(2)]
            for k in range(3)]
    diag3_s = [sb(f"diag3_s{i}", (128, 3, 128), bf16) for i in range(2)]
    fyw_s = [sb(f"fyw_s{i}", (3, 128), bf16) for i in range(2)]
    fyr_s = [sb(f"fyr_s{i}", (3, 128), bf16) for i in range(2)]

    pz = [psum(f"pz{i}", (128, H), f32) for i in range(3)]
    pa = [psum(f"pa{i}", (128, H), f32) for i in range(4)]

    class Sem:
        def __init__(self, name):
            self.h = es.enter_context(nc.semaphore(name))
            self.n = 0
        def inc(self, k):
            self.n += k
            return (self.h, self.n)
        def now(self):
            return (self.h, self.n)

    class Ring:
        def __init__(self, name, n):
            self.sems = [Sem(f"{name}{i}") for i in range(n)]
            self.nslots = n
        def write(self, slot, k=16):
            s = self.sems[slot % self.nslots]
            return s.inc(k)
        def last(self, slot):
            s = self.sems[slot % self.nslots]
            return (s.h, s.n)
        def all(self):
            return [(s.h, s.n) for s in self.sems]

    def wait_all(engine, ring):
        for sv in ring.all():
            wait(engine, sv)

    s_in = Sem("s_in")
    s_gprep = Sem("s_gprep")
    s_iprep = Sem("s_iprep")
    s_cc = Sem("s_cc")
    s_kpe = Sem("s_kpe"); s_kact = Sem("s_kact"); s_kmax = Sem("s_kmax")
    s_wn = Sem("s_wn"); s_wrap = Sem("s_wrap")
    s_dg = Sem("s_dg")
    s_ipe = Sem("s_ipe"); s_fy = Sem("s_fy")
    s_zpe = Sem("s_zpe")
    s_zact = Sem("s_zact"); s_ape = Sem("s_ape"); s_aact = Sem("s_aact")

    Q = {e: [] for e in ("sync", "tensor", "vector", "scalar", "gpsimd")}
    checkpoints = []
    def checkpoint():
        checkpoints.append({e: len(Q[e]) for e in Q})
    def emit(engine, fn):
        Q[engine].append(fn)
    def wait(engine, semv):
        s, v = semv
        if v > 0:
            emit(engine, lambda e, s=s, v=v: e.wait_ge(s, v))

    r_gk = Ring("r_gk", 2)     # interp table gathers (per gk buf)
    r_xt = Ring("r_xt", 2)     # xT3 tile loads
    r_h0 = Ring("r_h0", 2)     # h0T tile loads
    r_fyw = Ring("r_fyw", 2)   # finey dram writes
    r_fyr = Ring("r_fyr", 2)   # finey tile loads
    r_hT = Ring("r_hT", 2)     # transpose loads (per hT buf)
    r_g = Ring("r_g", GB)      # agg own gathers (per gath buf)
    r_gp = Ring("r_gp", GB)    # agg peer gathers (per gath buf)
    r_sg = Ring("r_sg", GB)    # combined S+idx loads
    r_gw = Ring("r_gw", 4)     # g_half dram writes (per gsb buf)
    r_hw = Ring("r_hw", 4)     # h dram writes (per hsb buf)
    r_ow = Ring("r_ow", 2)     # out writes (per osb buf)

    # ============ input loads ============
    loads = [
        (cxT3_s[:], "cxT3"), (negf2_s[:], "negf2"),
        (W0_s[:], "W0"), (W1_s[:], "W1"), (W2_s[:], "W2"),
        (W3a_s[:], "W3a"), (W3b_s[:], "W3b"), (W4_s[:], "W4"), (W5_s[:], "W5"),
        (brows_s[:], "brows"), (ones1_s[:], "ones1"), (identf_s[:], "identf"),
    ]
    for dst, srcn in loads:
        sm = s_in.inc(16)
        emit("sync", lambda e, d=dst, s=srcn, sm=sm: e.dma_start(
            out=d, in_=Din[s][:]).then_inc(sm[0], 16))
    IN_ALL = s_in.now()
    checkpoint()   # 0: loads

    # ============ KNN selection ============
    wait("tensor", IN_ALL)
    wait("scalar", IN_ALL)
    wait("vector", IN_ALL)
    NQ = NCPAD // 512
    for t in range(NTH):
        if t >= 2:
            wait("sync", (s_kpe.h, NQ * (t - 1)))
        sm = r_xt.write(t)
        emit("sync", lambda e, t=t, sm=sm: e.dma_start(
            out=xt_s[t % 2][:], in_=Din["xT3"][:, t * 128:(t + 1) * 128]
        ).then_inc(sm[0], 16))
        wait("tensor", r_xt.last(t))
        for q in range(NQ):
            gq = NQ * t + q
            if gq >= 3:
                wait("tensor", (s_kact.h, gq - 2))
            sm = s_kpe.inc(1)
            emit("tensor", lambda e, t=t, q=q, gq=gq, sm=sm: e.matmul(
                pz[gq % 3][:, 0:512], xt_s[t % 2][:],
                cxT3_s[:, q * 512:(q + 1) * 512],
                start=True, stop=True).then_inc(sm[0], 1))
        for q in range(NQ):
            gq = NQ * t + q
            wait("scalar", (s_kpe.h, gq + 1))
            if t >= 2 and q == 0:
                wait("scalar", (s_kmax.h, t - 1))
            sm = s_kact.inc(1)
            emit("scalar", lambda e, t=t, q=q, gq=gq, sm=sm: e.activation(
                nd2_s[t % 2][:, q * 512:(q + 1) * 512], pz[gq % 3][:, 0:512],
                ACTF.Identity, bias=negf2_s[:, t:t + 1], scale=1.0
            ).then_inc(sm[0], 1))
        wait("vector", (s_kact.h, NQ * (t + 1)))
        emit("vector", lambda e, t=t: e.max(bm_s[:, :, t], nd2_s[t % 2][:]))
        emit("vector", lambda e: e.drain())
        emit("vector", lambda e, t=t: e.max_index(
            bi_s[:, :, t], bm_s[:, :, t], nd2_s[t % 2][:]))
        sm = s_kmax.inc(1)
        emit("vector", lambda e, sm=sm: e.drain().then_inc(sm[0], 1))

    checkpoint()   # 1: knn select
    # weights on DVE
    emit("vector", lambda e: e.tensor_scalar(
        out=d2c_s[:], in0=bm_s[:, 0:3, :], scalar1=-1.0, scalar2=1e-16,
        op0=ALU.mult, op1=ALU.max))
    emit("vector", lambda e: e.drain())
    emit("vector", lambda e: e.reciprocal(w_s[:], d2c_s[:]))
    emit("vector", lambda e: e.drain())
    emit("vector", lambda e: e.tensor_reduce(
        out=wsum_s[:], in_=bass.AP(w_s, 0, [[3 * NTH, 128], [1, NTH], [NTH, 3]]),
        axis=mybir.AxisListType.X, op=ALU.add))
    emit("vector", lambda e: e.drain())
    emit("vector", lambda e: e.reciprocal(rs_s[:], wsum_s[:]))
    emit("vector", lambda e: e.drain())
    emit("vector", lambda e: e.tensor_tensor(
        out=wnb_s[:], in0=w_s[:],
        in1=bass.AP(rs_s, 0, [[NTH, 128], [0, 3], [1, NTH]]),
        op=ALU.mult))
    sm = s_wn.inc(1)
    emit("vector", lambda e, sm=sm: e.drain().then_inc(sm[0], 1))

    # wrapped idx build (gpsimd)
    wait("gpsimd", (s_kmax.h, NTH))
    for k in range(3):
        for g in range(8):
            sm = s_wrap.inc(16)
            emit("gpsimd", lambda e, k=k, g=g, sm=sm: e.dma_start(
                out=wrap_s[0:16, k, :, g],
                in_=bi_s[16 * g:16 * (g + 1), k, :],
            ).then_inc(sm[0], 16))
    wait("gpsimd", s_wrap.now())
    for rep in range(1, 8):
        sm = s_wrap.inc(16)
        emit("gpsimd", lambda e, rep=rep, sm=sm: e.dma_start(
            out=wrap_s[16 * rep:16 * (rep + 1)],
            in_=wrap_s[0:16],
        ).then_inc(sm[0], 16))
    WRAP_ALL = s_wrap.now()

    # interp
    wait("gpsimd", WRAP_ALL)
    wait("vector", s_wn.now())
    n_ich = (NTH + ICH - 1) // ICH
    for c in range(n_ich):
        t0 = c * ICH
        ntile = min(ICH, NTH - t0)
        for k in range(3):
            sm = r_gk.write(c)
            smp = s_iprep.inc(1)
            emit("gpsimd", lambda e, k=k, c=c, t0=t0, nt=ntile, sm=sm, smp=smp:
                 e.dma_gather(
                     out_ap=gk_s[k][c % 2][:, 0:nt, :],
                     in_ap=Din["ctab"][:],
                     idxs_ap=wrap_s[:, k, t0:t0 + nt, :].bitcast(i16),
                     num_idxs=nt * 128, num_idxs_reg=nt * 128,
                     elem_size=128,
                     prepare_only=True, sem=r_gk.sems[c % 2].h,
                     queue_num=c % 2,
                 ).then_inc(smp[0], 1))
        wait("gpsimd", s_iprep.now())
        if c >= 2:
            wait("gpsimd", (s_ipe.h, (c - 1) * ICH))
        emit("gpsimd", lambda e, c=c: e.trigger_dma(
            count=3, queue_num=c % 2))
        GK_NOW = r_gk.last(c)
        for tt in range(ntile):
            t = t0 + tt
            if t >= 2:
                wait("vector", (s_ipe.h, t - 1))
            emit("vector", lambda e, t=t: e.tensor_tensor(
                out=diag3_s[t % 2][:],
                in0=bass.AP(identf_s, 0, [[128, 128], [0, 3], [1, 128]]),
                in1=bass.AP(wnb_s, t, [[3 * NTH, 128], [NTH, 3], [0, 128]]),
                op=ALU.mult))
            sm = s_dg.inc(1)
            emit("vector", lambda e, sm=sm: e.drain().then_inc(sm[0], 1))
            wait("tensor", GK_NOW)
            wait("tensor", (s_dg.h, s_dg.n))
            if t >= 4:
                wait("tensor", (s_fy.h, t - 3))    # psum WAR
            for k in range(3):
                sm = s_ipe.inc(1) if k == 2 else None
                def mk_interp(t=t, tt=tt, k=k, c=c, sm=sm):
                    def f(e):
                        ins = e.matmul(
                            pa[t % 4][:, 0:128], gk_s[k][c % 2][:, tt, :],
                            diag3_s[t % 2][:, k, :],
                            start=(k == 0), stop=(k == 2))
                        if sm:
                            ins.then_inc(sm[0], 1)
                    return f
                emit("tensor", mk_interp())
            wait("scalar", (s_ipe.h, s_ipe.n))
            wait("scalar", r_fyw.last(t))
            sm = s_fy.inc(1)
            emit("scalar", lambda e, t=t, sm=sm: e.activation(
                fyw_s[t % 2][:], pa[t % 4][0:3, 0:128],
                ACTF.Copy, bias=0.0, scale=1.0).then_inc(sm[0], 1))
            sm = r_fyw.write(t)
            emit("scalar", lambda e, t=t, sm=sm: e.dma_start(
                out=fy_d[:, t * 128:(t + 1) * 128],
                in_=fyw_s[t % 2][:]).then_inc(sm[0], 16))
    FY_ALL = s_fy.now()
    KACT_ALL = s_kact.now()
    checkpoint()   # 2: interp

    # ============ GCN layers ============
    WCH = {1: W1_s, 2: W2_s, 3: W3a_s, 4: W4_s, 5: W5_s}
    layer_state = []   # per layer: rg/rgp/cc snapshots + h-write marks
    LAG = 20           # dense li+1 tile lag behind agg li tiles

    def new_dense_state(li, hmarks_src):
        L = LAYERS[li]
        return dict(
            li=li, L=L, width=128 if L["e2"] else H,
            ghalf=g2_half if L["e2"] else g_half[li % 2],
            gfull=g2_full if L["e2"] else g_full[li % 2],
            war=layer_state[li - 2] if (li >= 2 and not L["e2"]) else None,
            hmarks_src=hmarks_src,
            zpe_base=s_zpe.n, zact_base=s_zact.n,
            range_zpe=[], gw_marks=[])

    def dense_epilogue(st, t):
        width, ghalf, war = st["width"], st["ghalf"], st["war"]
        wait("vector", (s_zpe.h, st["zpe_base"] + t + 1))
        wait("vector", r_gw.last(t))
        sm = s_zact.inc(1)
        emit("vector", lambda e, t=t, w=width: e.tensor_scalar(
            out=gsb_s[t % 4][:, 0:w], in0=pz[t % 3][:, 0:w],
            scalar1=1.0, scalar2=None, op0=ALU.mult))
        emit("vector", lambda e, sm=sm: e.drain().then_inc(sm[0], 1))
        wait("scalar", (s_zact.h, s_zact.n))
        if t == 0 and war is not None:
            wait("scalar", war["cc"])         # WAR vs AG reads (li-2)
            for sv in war["rg"]:              # WAR vs own gathers (li-2)
                wait("scalar", sv)
        sm = r_gw.write(t)
        emit("scalar", lambda e, t=t, gd=ghalf, w=width, sm=sm: e.dma_start(
            out=gd[t * 128:(t + 1) * 128, :],
            in_=gsb_s[t % 4][:, 0:w]).then_inc(sm[0], 16))
        st["gw_marks"].append(r_gw.all())

    def dense_tile(st, t):
        """Emit dense work for tile t of layer st['li'] (li >= 1)."""
        li, L, width = st["li"], st["L"], st["width"]
        Wl = WCH[li]
        ri = t // RANGE_T
        rt0 = ri * RANGE_T
        rnt = min(RANGE_T, NTH - rt0)
        tt = t - rt0
        if tt == 0:
            wait("sync", (s_zpe.h,
                          st["zpe_base"] if ri < 2
                          else st["range_zpe"][ri - 2]))
            for sv in st["hmarks_src"][rt0 + rnt - 1]:   # h tiles ready
                wait("sync", sv)
            for cch in range(4):
                sm = r_hT.write(ri)
                emit("sync", lambda e, ri=ri, rt0=rt0, rnt=rnt, c=cch, sm=sm:
                     e.dma_start_transpose(
                         hT_s[ri % 2][:, c, 0:rnt * 128],
                         h_d[rt0 * 128:(rt0 + rnt) * 128,
                             c * 128:(c + 1) * 128],
                     ).then_inc(sm[0], 16))
            if li == 3 and ri == 0:
                wait_all("sync", r_fyw)
            wait("tensor", r_hT.last(ri))
        if L["fy"]:
            if t >= 2:
                wait("sync", (s_zpe.h, st["zpe_base"] + t - 1))
            sm = r_fyr.write(t)
            emit("sync", lambda e, t=t, sm=sm: e.dma_start(
                out=fyr_s[t % 2][:],
                in_=fy_d[:, t * 128:(t + 1) * 128]).then_inc(sm[0], 16))
        wait("tensor", (s_zact.h,
                        st["zact_base"] if t < 3
                        else st["zact_base"] + t - 2))
        for cch in range(4):
            last = (cch == 3) and not L["fy"]
            sm = s_zpe.inc(1) if last else None
            def mk_dense(t=t, tt=tt, ri=ri, cch=cch, Wl=Wl,
                         w=width, last=last, sm=sm):
                def f(e):
                    ins = e.matmul(
                        pz[t % 3][:, 0:w],
                        hT_s[ri % 2][:, cch, tt * 128:(tt + 1) * 128],
                        Wl[:, cch, 0:w],
                        start=(cch == 0), stop=last)
                    if sm:
                        ins.then_inc(sm[0], 1)
                return f
            emit("tensor", mk_dense())
        if L["fy"]:
            wait("tensor", r_fyr.last(t))
            sm = s_zpe.inc(1)
            emit("tensor", lambda e, t=t, sm=sm: e.matmul(
                pz[t % 3][:, 0:H], fyr_s[t % 2][:],
                W3b_s[:], start=False, stop=True).then_inc(sm[0], 1))
        if tt == rnt - 1:
            st["range_zpe"].append(s_zpe.n)
        dense_epilogue(st, t)

    def emit_ag_chunk(st, ci):
        """AllGather chunk ci for layer st['li'] (needs dense tiles < c1)."""
        c0, c1 = AGCH[ci]
        width, ghalf, gfull = st["width"], st["ghalf"], st["gfull"]
        li = st["li"]
        for sv in st["gw_marks"][c1 - 1]:
            wait("gpsimd", sv)
        if ci == 0 and li >= 2 and not st["L"]["e2"]:
            for sv in layer_state[li - 2]["rgp"]:   # WAR vs peer gathers
                wait("gpsimd", sv)
        rows = (c1 - c0) * 128
        sm = s_cc.inc(1)
        emit("gpsimd", lambda e, hh=ghalf, ff=gfull, c0=c0, rows=rows,
             w=width, sm=sm: e.collective_compute(
                 "AllGather",
                 ALU.bypass,
                 replica_groups=RG_PAIRS,
                 ins=[bass.AP(hh, c0 * 128 * w,
                              [[w, rows], [1, w]]).opt()],
                 outs=[bass.AP(ff, 2 * c0 * 128 * w,
                               [[w, 2 * rows], [1, w]]).opt()],
             ).then_inc(sm[0], 1))

    # ---------- dense layer 0 (standalone, from h0T) ----------
    dstates = [new_dense_state(0, None)]
    st0 = dstates[0]
    wait("tensor", KACT_ALL)      # pz WAR vs KNN ACT
    for t in range(NTH):
        if t >= 2:
            wait("sync", (s_zpe.h, st0["zpe_base"] + t - 1))
        sm = r_h0.write(t)
        emit("sync", lambda e, t=t, sm=sm: e.dma_start(
            out=h0_s[t % 2][:], in_=Din["h0T"][:, t * 128:(t + 1) * 128]
        ).then_inc(sm[0], 16))
        wait("tensor", r_h0.last(t))
        wait("tensor", (s_zact.h,
                        st0["zact_base"] if t < 3
                        else st0["zact_base"] + t - 2))
        sm = s_zpe.inc(1)
        emit("tensor", lambda e, t=t, sm=sm: e.matmul(
            pz[t % 3][:, 0:H], h0_s[t % 2][:],
            W0_s[:], start=True, stop=True).then_inc(sm[0], 1))
        dense_epilogue(st0, t)
        for ci, (c0, c1) in enumerate(AGCH):
            if t == c1 - 1:
                emit_ag_chunk(st0, ci)
    CC_NOW = s_cc.now()
    checkpoint()

    for li, L in enumerate(LAYERS):
        width = 128 if L["e2"] else H
        cur = dstates[li]
        ghalf, gfull = cur["ghalf"], cur["gfull"]

        # ---------- agg li, interleaving dense li+1 + its AG chunks ----------
        gbufs = gath2_s if L["e2"] else gath_s
        ape_base = s_ape.n
        aact_base = s_aact.n
        gprep_base = s_gprep.n
        hmarks = []
        nxt = None
        if li + 1 < len(LAYERS):
            nxt = new_dense_state(li + 1, hmarks)
            dstates.append(nxt)
        if li == 0:
            wait("tensor", (s_fy.h, NTH))   # pa WAR vs interp
        for t in range(NTH):
            po, pp, nsl = int(Po[t]), int(Pp[t]), int(P[t])
            # --- combined S+idx load (sync) ---
            wait("sync", (s_gprep.h,
                          gprep_base if t < GB
                          else gprep_base + 2 * (t - GB) + 2))
            wait("sync", (s_ape.h,
                          ape_base if t < GB else ape_base + t - GB + 1))
            sm = r_sg.write(t)
            emit("sync", lambda e, t=t, nsl=nsl, sm=sm: e.dma_start(
                out=sgt_s[t % GB][:, 0:nsl * 136],
                in_=Din["sgt"][:, SOFFC[t]:SOFFC[t] + nsl * 136]
            ).then_inc(sm[0], 16))
            # --- gather preps (gpsimd): own half + peer half ---
            wait("gpsimd", r_sg.last(t))
            smo = r_g.write(t)
            smp = s_gprep.inc(1)
            emit("gpsimd", lambda e, t=t, po=po, nsl=nsl, gh=ghalf, gb=gbufs,
                 w=width, smp=smp: e.dma_gather(
                     out_ap=gb[t % GB][:, 0:po, 0:w],
                     in_ap=gh[:],
                     idxs_ap=sgt_s[t % GB][:, nsl * 128:
                                           nsl * 128 + po * 8].bitcast(i16),
                     num_idxs=po * 128, num_idxs_reg=po * 128, elem_size=w,
                     single_packet=False,
                     prepare_only=True, sem=r_g.sems[t % GB].h,
                     queue_num=t % 2,
                 ).then_inc(smp[0], 1))
            smq = r_gp.write(t)
            smp = s_gprep.inc(1)
            emit("gpsimd", lambda e, t=t, po=po, pp=pp, nsl=nsl, gf=gfull,
                 gb=gbufs, w=width, smp=smp: e.dma_gather(
                     out_ap=gb[t % GB][:, po:po + pp, 0:w],
                     in_ap=gf[:],
                     idxs_ap=sgt_s[t % GB][:, nsl * 128 + po * 8:
                                           nsl * 136].bitcast(i16),
                     num_idxs=pp * 128, num_idxs_reg=pp * 128, elem_size=w,
                     single_packet=False,
                     prepare_only=True, sem=r_gp.sems[t % GB].h,
                     queue_num=2 + t % 2,
                 ).then_inc(smp[0], 1))
            wait("gpsimd", s_gprep.now())
            # own trigger: needs gath buf free + ALL own dense writes
            wait("gpsimd", (s_ape.h,
                            ape_base if t < GB else ape_base + t - GB + 1))
            if t == 0:
                for sv in cur["gw_marks"][NTH - 1]:
                    wait("gpsimd", sv)
            emit("gpsimd", lambda e, t=t: e.trigger_dma(
                count=1, queue_num=t % 2))
            # peer trigger: additionally needs the AllGather
            if t == 0:
                wait("gpsimd", CC_NOW)
            emit("gpsimd", lambda e, t=t: e.trigger_dma(
                count=1, queue_num=2 + t % 2))
            # --- matmuls (tensor): own slices, then peer, then bias ---
            wait("tensor", r_g.last(t))
            wait("tensor", (s_aact.h,
                            aact_base if t < 4 else aact_base + t - 3))
            for sl in range(po):
                emit("tensor", lambda e, t=t, sl=sl, gb=gbufs, w=width:
                     e.matmul(
                         pa[t % 4][:, 0:w],
                         sgt_s[t % GB][:, sl * 128:(sl + 1) * 128],
                         gb[t % GB][:, sl, 0:w],
                         start=(sl == 0), stop=False))
            wait("tensor", r_gp.last(t))
            for sl in range(po, nsl):
                emit("tensor", lambda e, t=t, sl=sl, gb=gbufs, w=width:
                     e.matmul(
                         pa[t % 4][:, 0:w],
                         sgt_s[t % GB][:, sl * 128:(sl + 1) * 128],
                         gb[t % GB][:, sl, 0:w],
                         start=False, stop=False))
            sm = s_ape.inc(1)
            emit("tensor", lambda e, t=t, li=li, w=width, sm=sm: e.matmul(
                pa[t % 4][:, 0:w], ones1_s[:],
                brows_s[:, li, 0:w], start=False, stop=True
            ).then_inc(sm[0], 1))
            # --- epilogue (scalar + sync) ---
            wait("scalar", (s_ape.h, s_ape.n))
            if L["e2"]:
                wait("scalar", r_ow.last(t))
            else:
                wait("scalar", r_hw.last(t))
            sm = s_aact.inc(1)
            if L["e2"]:
                emit("scalar", lambda e, t=t, sm=sm: e.activation(
                    osb_s[t % 2][:], pa[t % 4][:, 0:128], ACTF.Copy,
                    bias=0.0, scale=1.0).then_inc(sm[0], 1))
            else:
                emit("scalar", lambda e, t=t, sm=sm: e.activation(
                    hsb_s[t % 4][:], pa[t % 4][:, 0:H], ACTF.Relu,
                    bias=0.0, scale=1.0).then_inc(sm[0], 1))
            if L["e2"]:
                sm = r_ow.write(t)
                emit("scalar", lambda e, t=t, sm=sm: e.dma_start(
                    out=Din["out_nm"][t * 128:(t + 1) * 128, :],
                    in_=osb_s[t % 2][:]).then_inc(sm[0], 16))
            else:
                sm = r_hw.write(t)
                emit("scalar", lambda e, t=t, sm=sm: e.dma_start(
                    out=h_d[t * 128:(t + 1) * 128, :],
                    in_=hsb_s[t % 4][:]).then_inc(sm[0], 16))
            hmarks.append(r_hw.all())
            # --- interleave dense li+1 + its AG chunks ---
            if nxt is not None and t >= LAG:
                u = t - LAG
                dense_tile(nxt, u)
                for ci, (c0, c1) in enumerate(AGCH):
                    if u == c1 - 1:
                        emit_ag_chunk(nxt, ci)
        if nxt is not None:
            for u in range(NTH - LAG, NTH):
                dense_tile(nxt, u)
                for ci, (c0, c1) in enumerate(AGCH):
                    if u == c1 - 1:
                        emit_ag_chunk(nxt, ci)
            CC_NOW = s_cc.now()
        layer_state.append(dict(rg=r_g.all(), rgp=r_gp.all(),
                                cc=s_cc.now(), hmarks=hmarks))
        checkpoint()   # 3+li

    wait_all("sync", r_ow)
    wait_all("sync", r_hw)
    checkpoint()
    if PHASE < len(checkpoints):
        cut = checkpoints[PHASE]
        for e in Q:
            Q[e] = Q[e][:cut[e]]

    with nc.allow_non_contiguous_dma(reason="wrapped idx build"), \
            nc.Block() as block:
        @block.sync
        def _(e):
            for fn in Q["sync"]:
                fn(e)

        @block.tensor
        def _(e):
            for fn in Q["tensor"]:
                fn(e)

        @block.vector
        def _(e):
            for fn in Q["vector"]:
                fn(e)

        @block.scalar
        def _(e):
            for fn in Q["scalar"]:
                fn(e)

        @block.gpsimd
        def _(e):
            for fn in Q["gpsimd"]:
                fn(e)

    nc.finalize()
    return nc


# ================= host side =================

def host_prep(inputs):
    x = np.asarray(inputs["x"], np.float32)
    sdf = np.asarray(inputs["sdf"], np.float32)
    edge_index = np.asarray(inputs["edge_index"], np.int64)
    coarse_x = np.asarray(inputs["coarse_x"], np.float32)
    coarse_y = np.asarray(inputs["coarse_y"], np.float32)
    Ws = {k: np.asarray(inputs[k], np.float32) for k in (
        "pre_W0", "pre_W1", "pre_W2", "end_W0", "end_W1", "end_W2")}
    bs = {k: np.asarray(inputs[k], np.float32) for k in (
        "pre_b0", "pre_b1", "pre_b2", "end_b0", "end_b1", "end_b2")}

    cxT3 = np.zeros((3, NCPAD), np.float32)
    cxT3[0, :NC] = 2 * coarse_x[:, 0]
    cxT3[1, :NC] = 2 * coarse_x[:, 1]
    cxT3[2, :NC] = -(coarse_x[:, 0] ** 2 + coarse_x[:, 1] ** 2)
    cxT3[0, NC:] = 2e4; cxT3[1, NC:] = 2e4; cxT3[2, NC:] = -2e8

    brows = np.zeros((6, H), np.float32)
    for i, k in enumerate(("pre_b0", "pre_b1", "pre_b2", "end_b0", "end_b1")):
        brows[i] = bs[k]
    brows[5, :OUT] = bs["end_b2"]

    W5 = np.zeros((H, 128), np.float32)
    W5[:, :OUT] = Ws["end_W2"]

    def pmaj(w):   # [512, X] -> [128, 4, X]
        return np.ascontiguousarray(
            w.reshape(4, 128, w.shape[1]).transpose(1, 0, 2))

    common = dict(
        cxT3=cxT3,
        W0=Ws["pre_W0"].astype(bfnp),
        W1=pmaj(Ws["pre_W1"]).astype(bfnp),
        W2=pmaj(Ws["pre_W2"]).astype(bfnp),
        W3a=pmaj(Ws["end_W0"][OUT:]).astype(bfnp),
        W3b=Ws["end_W0"][:OUT].astype(bfnp),
        W4=pmaj(Ws["end_W1"]).astype(bfnp),
        W5=pmaj(W5).astype(bfnp),
        brows=brows.astype(bfnp)[None],
        ones1=np.ones((1, 128), bfnp),
        identf=np.eye(128, dtype=np.float32),
    )

    # ---- pass 1: per-sample node->slot assignment + per-core tile stats ----
    samples = []
    for s in range(B):
        xs = x[s * NF:(s + 1) * NF]
        e = edge_index[:, s * E_PER:(s + 1) * E_PER] - s * NF
        cy = coarse_y[s * NC:(s + 1) * NC]

        deg = np.bincount(e[1], minlength=NF).astype(np.float32) + 1.0
        dinv = (1.0 / np.sqrt(deg)).astype(np.float32)

        # balanced global tile assignment (snake over degree-sorted nodes)
        order = np.argsort(-deg, kind="stable")
        tile_seq = np.arange(NT)
        snake = np.concatenate([tile_seq, tile_seq[::-1]])
        bins = np.resize(snake, NF)
        gtile = np.empty(NF, np.int64)   # node -> global tile
        lane = np.empty(NF, np.int64)
        for t in range(NT):
            sel = np.where(bins == t)[0]
            gtile[order[sel]] = t
            lane[order[sel]] = np.arange(len(sel))

        # self loops ride in the own-source group
        e_aug = np.concatenate([e, np.stack([np.arange(NF)] * 2)], axis=1)

        # in-edge count per global tile (incl self loops)
        cin = np.bincount(gtile[e_aug[1]], minlength=NT)

        # per half: order local tiles by in-edge count desc
        half = (gtile >= NTH).astype(np.int64)
        ltile = np.empty(NF, np.int64)
        sco = np.zeros((2, NTH), np.int64)   # own-source slices per tile
        scp = np.zeros((2, NTH), np.int64)   # peer-source slices per tile
        for p in range(2):
            gts = np.arange(p * NTH, (p + 1) * NTH)
            perm = gts[np.argsort(-cin[gts], kind="stable")]
            inv = np.empty(NTH, np.int64)
            inv[perm - p * NTH] = np.arange(NTH)
            mask = half == p
            ltile[mask] = inv[gtile[mask] - p * NTH]
        src_half = half[e_aug[0]]
        dst_half = half[e_aug[1]]
        for p in range(2):
            emask = dst_half == p
            dt_ = ltile[e_aug[1][emask]]
            own = src_half[emask] == p
            sco[p] = np.bincount(dt_[own], minlength=NTH)   # own counts
            scp[p] = np.bincount(dt_, minlength=NTH)        # total counts

        # node -> row in g_full (chunk-major AllGather layout: chunk ci of
        # AGCH writes [even c0..c1 | odd c0..c1] at row 2*c0*128)
        nidg = np.zeros(NF, np.int64)
        for (c0, c1) in AGCH:
            m = (ltile >= c0) & (ltile < c1)
            nidg[m] = (2 * c0 * 128 + half[m] * ((c1 - c0) * 128)
                       + (ltile[m] - c0) * 128 + lane[m])
        # node -> row in concat(even out_nm, odd out_nm) (output assembly)
        nidl = half * NPADH + ltile * 128 + lane
        samples.append(dict(xs=xs, e=e_aug, cy=cy, dinv=dinv, half=half,
                            ltile=ltile, lane=lane, nidg=nidg, nidl=nidl,
                            sco=sco, scp=scp))

    # profiles: own group sized to the min own count over cores (zero own
    # padding; overflow spills into the peer group which reads g_full)
    own_min = np.full(NTH, 1 << 30, np.int64)
    tot_max = np.zeros(NTH, np.int64)
    for sm in samples:
        own_min = np.minimum(own_min, sm["sco"].min(axis=0))
        tot_max = np.maximum(tot_max, sm["scp"].max(axis=0))
    Po = np.maximum(own_min // 128, 1)
    Pp = np.maximum(np.ceil((tot_max - Po * 128) / 128).astype(np.int64), 1)
    P = Po + Pp
    assert P.max() <= 12, f"slice overflow {P.max()}"
    SOFF = np.concatenate([[0], np.cumsum(P)]).astype(int)
    SLOT_TOT = int(SOFF[-1]) * 128
    SOFFC = (SOFF * 136).astype(int)

    # ---- pass 2: per-core arrays ----
    in_maps, metas = [], []
    for s in range(B):
        smp_ = samples[s]
        xs, e, cy = smp_["xs"], smp_["e"], smp_["cy"]
        dinv, half, ltile, lane, nidg = (
            smp_["dinv"], smp_["half"], smp_["ltile"], smp_["lane"],
            smp_["nidg"])

        ctab = np.zeros((NCPAD, 128), np.float32)
        ctab[:NC, 0:OUT] = cy
        ctab = ctab.astype(bfnp)

        for p in range(2):
            own = half == p
            lrow = ltile * 128 + lane          # local row id (own nodes)

            grow = np.zeros(SLOT_TOT, np.int16)
            sT = np.zeros((SLOT_TOT, 128), np.float32)
            # two slot groups per tile: first Po[t]*128 own-source edges
            # (local g_half rows, incl self loops) at SOFF[t]*128; all
            # remaining edges (own overflow + peer sources, g_full rows)
            # at (SOFF[t]+Po[t])*128
            emask = half[e[1]] == p
            e_src, e_dst = e[0][emask], e[1][emask]
            is_own = half[e_src] == p
            ecol_t = ltile[e_dst]
            # order: per tile, own-source edges first
            o = np.lexsort((~is_own, ecol_t))
            e_src, e_dst, is_own = e_src[o], e_dst[o], is_own[o]
            ecol_t = ecol_t[o]
            ecol_l = lrow[e_dst] % 128
            ewt = dinv[e_src] * dinv[e_dst]
            tstart = np.searchsorted(ecol_t, np.arange(NTH))
            cnts = (np.searchsorted(ecol_t, np.arange(NTH), side="right")
                    - tstart)
            assert (cnts <= P * 128).all(), "profile overflow"
            rank = np.arange(len(ecol_t)) - np.repeat(tstart, cnts)
            in_own = rank < np.repeat(Po * 128, cnts)
            # own group must contain only own-source edges
            assert not (in_own & ~is_own).any(), "own group underfilled"
            slot = (SOFF[ecol_t] * 128 + rank).astype(np.int64)
            erow = np.where(in_own, lrow[e_src], nidg[e_src])
            grow[slot] = erow.astype(np.int16)
            sT[slot, ecol_l] = ewt

            # combined per-tile [S p-major | idx] tensor
            tmp = np.ascontiguousarray(grow.reshape(SLOT_TOT // 16, 16).T)
            growc = np.ascontiguousarray(np.tile(tmp, (8, 1)))  # [128, S/16]
            sgt = np.zeros((128, int(SOFFC[-1])), bfnp)
            for t in range(NTH):
                nsl = int(P[t]); base = int(SOFFC[t])
                blk = sT[SOFF[t] * 128:(SOFF[t] + nsl) * 128]
                pm = blk.reshape(nsl, 128, 128).transpose(1, 0, 2)
                sgt[:, base:base + nsl * 128] = (
                    pm.reshape(128, nsl * 128).astype(bfnp))
                gb = np.ascontiguousarray(
                    growc[:, SOFF[t] * 8:(SOFF[t] + nsl) * 8])
                sgt[:, base + nsl * 128:base + nsl * 136] = gb.view(bfnp)

            # node features / positions at local slots
            f01 = np.full((NPADH, 2), 1e3, np.float32)
            f01[lrow[own]] = xs[own][:, 0:2]
            xT3 = np.ones((3, NPADH), np.float32)
            xT3[0] = f01[:, 0]; xT3[1] = f01[:, 1]
            negf2 = np.ascontiguousarray(
                (-(f01[:, 0] ** 2 + f01[:, 1] ** 2)).reshape(NTH, 128).T)

            h0 = np.zeros((NPADH, 6), np.float32)
            h0[lrow[own], 0:D_IN] = xs[own]
            h0[lrow[own], D_IN] = sdf[own, 0]
            h0T = np.ascontiguousarray(h0.T).astype(bfnp)

            m = dict(common)
            m.update(xT3=xT3, negf2=negf2, h0T=h0T, sgt=sgt, ctab=ctab)
            in_maps.append(m)
        metas.append(smp_["nidl"])

    return in_maps, metas, (tuple(Po.tolist()), tuple(Pp.tolist()))


_prog_cache = {}


def kernel(**inputs):
    in_maps, metas, P = host_prep(inputs)
    if _prog_cache.get("P") != P:
        _prog_cache["nc"] = build_program(np.array(P[0]), np.array(P[1]))
        _prog_cache["P"] = P
    nc = _prog_cache["nc"]

    res = run_bass_kernel_spmd(nc, in_maps, list(range(N_CORES)))
    global _last_exec_ns, _last_trace
    _last_exec_ns = res.exec_time_ns
    _last_trace = res.instructions_and_trace

    out = np.empty((B * NF, OUT), np.float32)
    for s in range(B):
        full = np.concatenate([
            np.asarray(res.results[2 * s]["out_nm"]),
            np.asarray(res.results[2 * s + 1]["out_nm"]),
        ], axis=0)
        out[s * NF:(s + 1) * NF] = full[metas[s], 0:OUT]
    return out
